# revision 23
# baseline (speedup 1.0000x reference)
"""Trainium2 Bass kernel for nn_MixtureOfExperts (B=524288, IN=59, E=4, H=64).

Data-parallel over 8 cores (65536 rows each).  v2 design:

 - Host folds BN into weights/biases, collapses the embed head w3@wp -> wep
   (H->1 per expert), pre-transposes x into feature-major [feat, batch]
   fp16 layout (two independent 64-feature batch-halves on partition
   halves), and emits an fp8 (e4m3) hi/lo pair of x for the gating path.
 - Stage 1: fp16 matmuls, one per expert-pair (M=128 = 2 experts x H=64),
   biases folded via the ones-row of x.
 - Stage 2: fp16 block-diagonal matmuls (K=128 = 2 experts' h1 features,
   M=128 = 2 experts' h2) -- half the matmul count of per-expert K=64.
 - Gating hidden: fp8 DoubleRow matmuls (2 K-tiles: x8 and the scaled
   residual r8), 0.5 cycles/row; gw1 quantization noise only perturbs the
   softmax gates (~4e-3 final rel err).
 - preds / logits: tiny-N matmuls with h2 / g1 slices as the *stationary*
   operand and wep / gw2 as the moving operand -- cost is the output free
   size (2-4 columns) instead of a full 512-column pass.  Outputs land
   batch-major in one PSUM bank per q-group.
 - Tail (per q-group of 8 tiles): softmax-combine in batch-major layout
   with cheap strided DVE/Pool ops.
 - PSUM->SBUF evictions are spread across Pool/Act/DVE to balance engines.
"""

import numpy as np
import ml_dtypes

import concourse.bass as bass
import concourse.mybir as mybir
import concourse.tile as tile
from concourse import bacc
from concourse.bass_utils import run_bass_kernel_spmd

F32 = mybir.dt.float32
FP16 = mybir.dt.float16
FP8 = mybir.dt.float8e4
AF = mybir.ActivationFunctionType
ALU = mybir.AluOpType
DR = mybir.MatmulPerfMode.DoubleRow

B, IN, E, H, EMB, GH = 524288, 59, 4, 64, 32, 32
EPS = 1e-5
NCORES = 8
BC = B // NCORES          # 65536 rows per core
S = 8192                  # rows per batch-half per superstep
SUP = BC // (2 * S)       # 4 supersteps
NQ = S // (4 * 512)       # 4 q-groups per superstep
BT = 512

_CACHE = {}


def _build():
    nc = bacc.Bacc(trn_type="TRN2")
    x16_d = nc.dram_tensor("x16", (SUP, 128, S), FP16, kind="ExternalInput")
    x8_d = nc.dram_tensor("x8", (SUP, 128, 2 * S), FP8, kind="ExternalInput")
    w16_d = nc.dram_tensor("w16", (128, 692), FP16, kind="ExternalInput")
    w8_d = nc.dram_tensor("w8", (128, 1024), FP8, kind="ExternalInput")
    cst_d = nc.dram_tensor("cst", (128, 1024), F32, kind="ExternalInput")
    out_d = nc.dram_tensor("out", (SUP, 128, 128), F32, kind="ExternalOutput")

    with tile.TileContext(nc) as tc:
        with (
            tc.tile_pool(name="consts", bufs=1) as consts,
            tc.tile_pool(name="xp", bufs=2) as xp,
            tc.tile_pool(name="x8p", bufs=2) as x8p,
            tc.tile_pool(name="hs", bufs=6) as hs,
            tc.tile_pool(name="gs", bufs=3) as gs,
            tc.tile_pool(name="tl", bufs=4) as tl,
            tc.tile_pool(name="outp", bufs=2) as outp,
            tc.tile_pool(name="ps", bufs=1, space="PSUM") as ps,
        ):
            w16_sb = consts.tile([128, 692], FP16)
            nc.sync.dma_start(out=w16_sb, in_=w16_d[:, :])
            w8_sb = consts.tile([128, 1024], FP8)
            nc.sync.dma_start(out=w8_sb, in_=w8_d[:, :])
            cst_sb = consts.tile([128, 1024], F32)
            nc.sync.dma_start(out=cst_sb, in_=cst_d[:, :])

            w1p = w16_sb[:, 0:256]         # stage1 lhsT: pair01 | pair23
            w2b = w16_sb[:, 256:512]       # stage2 block-diag: pair01 | pair23
            wep = w16_sb[:, 512:516]       # preds moving cols (4)
            gw2z = w16_sb[:, 516:532]      # logits moving cols, zero-padded
            biascol = w16_sb[:, 532:564]   # bias-mm moving cols
            e0row = w16_sb[:, 564:692]     # bias-mm lhsT (row0 = 1)
            w8v = w8_sb.rearrange("p (g two m) -> p g two m", g=4, two=2)
            c2aw = cst_sb[:, 0:1]          # stage2 bias, pair01
            c2bw = cst_sb[:, 512:513]      # stage2 bias, pair23
            ones8 = cst_sb[:, 516:524]     # 1.0 x 8 (for reciprocal)

            GPQ = 4                       # 2-tile groups per q-group
            TOT = SUP * NQ * GPQ          # total groups
            grp = {}                      # live pipeline state per group
            perq = {}                     # (k, q) -> {PL, g1a, g1b}
            perk = {}                     # k -> {x16, x8v, oacc}
            GT = [[(0, 0), (0, 1)], [(0, 2), (0, 3)],
                  [(1, 0), (1, 1)], [(1, 2), (1, 3)]]

            def load_ss(k):
                x16_sb = xp.tile([128, S], FP16, tag="x16")
                for ch in range(4):
                    cw = S // 4
                    nc.sync.dma_start(
                        out=x16_sb[:, ch * cw : (ch + 1) * cw],
                        in_=x16_d[k][:, ch * cw : (ch + 1) * cw])
                x8_sb = x8p.tile([128, 2 * S], FP8, tag="x8")
                for ch in range(4):
                    cw = 2 * S // 4
                    nc.sync.dma_start(
                        out=x8_sb[:, ch * cw : (ch + 1) * cw],
                        in_=x8_d[k][:, ch * cw : (ch + 1) * cw])
                perk[k] = {"x16": x16_sb,
                           "x8v": x8_sb.rearrange("p (two s) -> p two s",
                                                  two=2)}

            def gate_round(k, q, half):
                gw = ps.tile([128, BT], F32, tag="gw", bufs=1, name="gw")
                base = 64 * half
                for gi in range(4):
                    c0 = (4 * q + gi) * BT
                    nc.tensor.matmul(
                        out=gw,
                        lhsT=w8v[base : base + 64, gi],
                        rhs=perk[k]["x8v"][base : base + 64, :,
                                           c0 : c0 + BT],
                        start=(gi == 0), stop=(gi == 3),
                        perf_mode=DR, skip_group_check=True)
                g1 = gs.tile([128, BT], FP16, tag="g1")
                nc.scalar.activation(g1, gw, AF.Relu, scale=1.0 / 16.0)
                perq[(k, q)]["g1a" if half == 0 else "g1b"] = g1

            def emit_s1(g):
                k, q, j = g // (NQ * GPQ), (g // GPQ) % NQ, g % GPQ
                st = {"k": k, "q": q, "tiles": GT[j], "h1s": []}
                grp[g] = st
                for half, pi in GT[j]:
                    c0 = (4 * q + pi) * BT
                    base = 64 * half
                    xs = perk[k]["x16"][base : base + 64, c0 : c0 + BT]
                    h1 = ps.tile([128, 2 * BT], F32, tag="w", bufs=3,
                                 name="h1")
                    nc.tensor.matmul(
                        out=h1[:, 0:BT],
                        lhsT=w1p[base : base + 64, 0:128],
                        rhs=xs, start=True, stop=True,
                        skip_group_check=True)
                    nc.tensor.matmul(
                        out=h1[:, BT : 2 * BT],
                        lhsT=w1p[base : base + 64, 128:256],
                        rhs=xs, start=True, stop=True,
                        skip_group_check=True)
                    h1s = hs.tile([128, 2 * BT], FP16, tag="h1s")
                    if len(st["h1s"]) == 0:
                        nc.scalar.activation(h1s, h1, AF.Relu)
                    else:
                        nc.vector.tensor_scalar(
                            h1s, h1, 0.0, None, ALU.max)
                    st["h1s"].append(h1s)

            def emit_s2(g):
                st = grp[g]
                h2A = ps.tile([128, 2 * BT], F32, tag="w", bufs=3,
                              name="h2A")
                h2B = ps.tile([128, 2 * BT], F32, tag="w", bufs=3,
                              name="h2B")
                for i in range(2):
                    cc = i * BT
                    nc.tensor.matmul(
                        out=h2A[:, cc : cc + BT], lhsT=w2b[:, 0:128],
                        rhs=st["h1s"][i][:, 0:BT], start=True, stop=True,
                        skip_group_check=True)
                for i in range(2):
                    cc = i * BT
                    nc.tensor.matmul(
                        out=h2B[:, cc : cc + BT], lhsT=w2b[:, 128:256],
                        rhs=st["h1s"][i][:, BT : 2 * BT], start=True,
                        stop=True, skip_group_check=True)
                sa = hs.tile([128, 2 * BT], FP16, tag="h2sa")
                nc.scalar.activation(sa, h2A, AF.Relu, bias=c2aw)
                sb = hs.tile([128, 2 * BT], FP16, tag="h2sb")
                nc.vector.tensor_scalar(
                    sb, h2B, c2bw, 0.0, ALU.add, ALU.max)
                st["sa"], st["sb"] = sa, sb

            def emit_tinies(g):
                st = grp[g]
                k, q = st["k"], st["q"]
                pq = perq[(k, q)]
                if "PL" not in pq:
                    pq["PL"] = ps.tile([128, 256], F32, tag="pl", bufs=1,
                                       name="PL")
                PL = pq["PL"]
                j = g % GPQ
                for i, (half, pi) in enumerate(st["tiles"]):
                    tm = 2 * j + i
                    j0 = 32 * tm
                    cc = i * BT
                    sa, sb = st["sa"], st["sb"]
                    g1x = pq["g1a"] if half == 0 else pq["g1b"]
                    nc.tensor.matmul(
                        out=PL[:, j0 : j0 + 32], lhsT=e0row,
                        rhs=biascol, start=True, stop=False,
                        skip_group_check=True)
                    for s in range(4):
                        sl = slice(cc + 128 * s, cc + 128 * s + 128)
                        sg = slice(128 * s, 128 * s + 128)
                        nc.tensor.matmul(
                            out=PL[:, j0 + 4 * s : j0 + 4 * s + 2],
                            lhsT=sa[:, sl], rhs=wep[:, 0:2],
                            start=False, stop=False,
                            skip_group_check=True)
                        nc.tensor.matmul(
                            out=PL[:, j0 + 4 * s + 2 : j0 + 4 * s + 4],
                            lhsT=sb[:, sl], rhs=wep[:, 2:4],
                            start=False, stop=False,
                            skip_group_check=True)
                        nc.tensor.matmul(
                            out=PL[:, j0 + 16 + 4 * s : j0 + 20 + 4 * s],
                            lhsT=g1x[:, sg],
                            rhs=gw2z[:, 4 * pi : 4 * pi + 4],
                            start=False, stop=(s == 3),
                            skip_group_check=True)
                del grp[g]

            def emit_tail(k, q):
                PL = perq[(k, q)]["PL"]
                PLv = PL.rearrange("p (t j) -> p t j", t=8)
                EL = tl.tile([128, 128], FP16, tag="el")
                nc.scalar.activation(
                    EL.rearrange("p (t j) -> p t j", t=8),
                    PLv[:, :, 16:32], AF.Exp)
                W = tl.tile([128, 128], FP16, tag="w")
                nc.vector.tensor_mul(
                    W.rearrange("p (t j) -> p t j", t=8),
                    PLv[:, :, 0:16],
                    EL.rearrange("p (t j) -> p t j", t=8))
                Wv = W.rearrange("p (n e) -> p n e", e=4)
                ELv = EL.rearrange("p (n e) -> p n e", e=4)
                n1 = tl.tile([128, 64], F32, tag="n1")
                n1v = n1.rearrange("p (n e) -> p n e", e=2)
                nc.gpsimd.tensor_add(n1v, Wv[:, :, 0:2], Wv[:, :, 2:4])
                d1 = tl.tile([128, 64], F32, tag="d1")
                d1v = d1.rearrange("p (n e) -> p n e", e=2)
                nc.gpsimd.tensor_add(d1v, ELv[:, :, 0:2], ELv[:, :, 2:4])
                num = tl.tile([128, 32], F32, tag="num")
                nc.gpsimd.tensor_add(
                    num.rearrange("p (n e) -> p n e", e=1),
                    n1v[:, :, 0:1], n1v[:, :, 1:2])
                den = tl.tile([128, 32], F32, tag="den")
                nc.gpsimd.tensor_add(
                    den.rearrange("p (n e) -> p n e", e=1),
                    d1v[:, :, 0:1], d1v[:, :, 1:2])
                rec = tl.tile([128, 32], F32, tag="rec")
                nc.vector.reciprocal(rec, den)
                if "oacc" not in perk[k]:
                    perk[k]["oacc"] = outp.tile([128, 128], F32,
                                                tag="oacc", name="oacc")
                nc.gpsimd.tensor_mul(
                    perk[k]["oacc"][:, 32 * q : 32 * q + 32], num, rec)
                if q == NQ - 1:
                    nc.sync.dma_start(out=out_d[k], in_=perk[k]["oacc"])

            for g in range(TOT + 3):
                if g < TOT:
                    k = g // (NQ * GPQ)
                    q = (g // GPQ) % NQ
                    j = g % GPQ
                    if j == 0:
                        if q == 0:
                            load_ss(k)
                        perq[(k, q)] = {}
                        gate_round(k, q, 0)
                    if j == 2:
                        gate_round(k, q, 1)
                    emit_s1(g)
                if 1 <= g < TOT + 1:
                    emit_s2(g - 1)
                if 3 <= g < TOT + 3:
                    gg = g - 3
                    emit_tinies(gg)
                    if gg % GPQ == GPQ - 1:
                        emit_tail(gg // (NQ * GPQ), (gg // GPQ) % NQ)

    if not nc.is_finalized():
        nc.finalize()
    return nc


def _q8(a, scale):
    return np.asarray(a * scale, np.float32).astype(ml_dtypes.float8_e4m3)


def _pack_host(w1, b1, bn1_g, bn1_b, bn1_m, bn1_v, w2, b2, bn2_g, bn2_b,
               bn2_m, bn2_v, w3, b3, wp, bp, gw1, gb1, gw2, gb2):
    f = np.float32
    s1 = (bn1_g / np.sqrt(bn1_v + EPS)).astype(f)              # (E,H)
    w1e = (w1 * s1[:, None, :]).astype(f)                       # (E,IN,H)
    c1 = ((b1 - bn1_m) * s1 + bn1_b).astype(f)                  # (E,H)
    s2 = (bn2_g / np.sqrt(bn2_v + EPS)).astype(f)
    w2e = (w2 * s2[:, None, :]).astype(f)                       # (E,H,H)
    c2 = ((b2 - bn2_m) * s2 + bn2_b).astype(f)                  # (E,H)
    wepv = np.einsum("ehm,em->eh", w3, wp).astype(f)            # (E,H)
    bep = (np.einsum("em,em->e", b3, wp) + bp).astype(f)        # (E,)

    # ---- fp16 block [128, 692]
    w16d = np.zeros((128, 692), f)
    for pr in range(2):                       # stage1 lhsT, expert pairs
        for j in range(2):
            e = 2 * pr + j
            for half in range(2):
                r0 = 64 * half
                w16d[r0 : r0 + IN, 128 * pr + 64 * j : 128 * pr + 64 * j + 64] = w1e[e]
                w16d[r0 + IN, 128 * pr + 64 * j : 128 * pr + 64 * j + 64] = c1[e]
    for pr in range(2):                       # stage2 block-diag lhsT
        e0, e1 = 2 * pr, 2 * pr + 1
        blk = np.zeros((128, 128), f)
        blk[0:64, 0:64] = w2e[e0]
        blk[64:128, 64:128] = w2e[e1]
        w16d[:, 256 + 128 * pr : 384 + 128 * pr] = blk
    wepp = np.zeros((128, 4), f)              # preds moving cols
    wepp[0:64, 0] = wepv[0]
    wepp[64:128, 1] = wepv[1]
    wepp[0:64, 2] = wepv[2]
    wepp[64:128, 3] = wepv[3]
    w16d[:, 512:516] = wepp
    for g in range(4):                # logits moving cols, zero-padded slots
        w16d[32 * g : 32 * g + 32, 516 + 4 * g : 520 + 4 * g] = gw2
    pat = np.zeros(32, f)                     # bias-mm moving cols (row 0)
    for s in range(4):
        for e in range(4):
            pat[4 * s + e] = bep[e]
            pat[16 + 4 * s + e] = gb2[e]
    w16d[0, 532:564] = pat
    w16d[0, 564:692] = 1.0                    # e0row lhsT: row0 = ones
    w16d = w16d.astype(np.float16)

    # ---- fp8 gate lhsT [128, 1024] = [p, slot g, ktile, 128]
    w8 = np.zeros((64, 4, 2, 128), np.float32)
    for g in range(4):
        w8[:IN, g, 0, 32 * g : 32 * g + 32] = 16.0 * gw1
        w8[:IN, g, 1, 32 * g : 32 * g + 32] = 2.0 * gw1
        w8[IN, g, 0, 32 * g : 32 * g + 32] = 16.0 * gb1
    w8 = np.concatenate([w8, w8], axis=0).reshape(128, 1024)
    w8 = w8.astype(ml_dtypes.float8_e4m3)

    # ---- f32 consts [128, 1024]: c2 wide bias (pair01 | pair23)
    cst = np.zeros((128, 1024), f)
    cst[:, 0:512] = np.concatenate([c2[0], c2[1]])[:, None]
    cst[:, 512:1024] = np.concatenate([c2[2], c2[3]])[:, None]
    cst[:, 516:524] = 1.0
    return dict(w16=np.ascontiguousarray(w16d),
                w8=np.ascontiguousarray(w8),
                cst=np.ascontiguousarray(cst))


def _prep_x_core(xc):
    """xc: (BC, 59) f32 -> x16 [SUP,128,S] fp16, x8 [SUP,128,2S] fp8."""
    xt = np.zeros((64, BC), np.float32)
    xt[:IN] = xc.T
    xt[IN] = 1.0
    # [64, BC] -> [SUP, 2(half), 64, S] -> [SUP, 128, S]
    xq = xt.reshape(64, SUP, 2, S).transpose(1, 2, 0, 3).reshape(SUP, 128, S)
    x16 = np.ascontiguousarray(xq).astype(np.float16)
    x8 = xq.astype(ml_dtypes.float8_e4m3)
    r = xq - x8.astype(np.float32)
    r8 = _q8(r, 8.0)
    # kill the residual of the ones-row (row 59 within each 64-block)
    r8.reshape(SUP, 2, 64, S)[:, :, IN:, :] = 0
    x8c = np.concatenate([x8, r8], axis=2)          # [SUP, 128, 2S]
    return x16, np.ascontiguousarray(x8c)


def _unpack_out(o):
    """o: [SUP, 128, 128] f32 -> (BC,) f32."""
    # col = 32q + 4tm + s ; tm = 2j+i, (half, pi) = (j//2, 2*(j%2)+i)
    v = o.reshape(SUP, 128, 4, 2, 2, 2, 4)    # k, p, q, j1, j2, i, s
    v = v.transpose(0, 3, 2, 4, 5, 6, 1)      # k, half, q, j2, i, s, p
    return v.reshape(BC)


def kernel(**inputs):
    x = np.asarray(inputs["x"], dtype=np.float32)
    wk = {kk: np.asarray(v, dtype=np.float32) for kk, v in inputs.items()
          if kk != "x"}
    packed = _pack_host(**wk)

    if "nc" not in _CACHE:
        _CACHE["nc"] = _build()
    nc = _CACHE["nc"]

    in_maps = []
    for c in range(NCORES):
        x16, x8 = _prep_x_core(x[c * BC : (c + 1) * BC])
        m = {"x16": x16, "x8": x8}
        m.update(packed)
        in_maps.append(m)

    res = run_bass_kernel_spmd(nc, in_maps, core_ids=list(range(NCORES)))
    _CACHE["last"] = res
    outs = [_unpack_out(r["out"]) for r in res.results]
    return np.concatenate(outs).reshape(B, 1).astype(np.float32)
